# revision 2
# baseline (speedup 1.0000x reference)
"""Trainium2 Bass kernel v2 for nn_CCPM — data-parallel over batch on 8 cores.

Per core (B_core = 2048, chunks of BC = 64 samples, gathered in pairs):
- embedding gather: per 128-sample pair, one int32 indirect DMA per feature
  (23 total) into row layout [128, F, D] — full-range indices, no quadrant
  redundancy, no masks ("ind1" mode; "quad" falls back to the baseline int16
  wrapped dma_gather + mask combine).
- PE transposes per h -> column layout [23, 128, 64]; convs as PE matmuls.
- bias + index-tiebreak (eta ladder) + sign/shift folded into the ScalarE
  PSUM->SBUF activation so topk sees strictly positive per-segment-unique
  values; relu/un-shift rides the next layer's flip copy:
    L1: rn1 = relu(1 - r1 + (i+1) eta)   (drop-8-smallest == kill-8-largest)
    L2: rp2 = relu(r2 + 1 + (24-i) eta)  (kill-7-largest on a scratch copy)
    L3: rp3 = relu(r3 + 1 + (9-i) eta)
- topk by iterated extraction: segmented max reduce (DVE) + is_equal mask
  (split DVE/GPSIMD by halves) + copy_predicated-to-zero (DVE). After k
  rounds the zeros mark the extracted set exactly.
- compaction: u8 drop mask + prefix scan + descending-shift predicated
  copies keyed on the raw inclusive count (imposters overwritten).
- L3 selection absorbed into the dense layer; h-reduction via a tiny PE
  matmul; sigmoid on ScalarE.
"""
import sys

for p in ("/opt/trn_rl_repo", "/root/.axon_site/_ro/trn_rl_repo"):
    if p not in sys.path:
        sys.path.append(p)

import numpy as np

import concourse.bacc as bacc
import concourse.bass as bass
import concourse.mybir as mybir
import concourse.tile as tile
from concourse.bass import IndirectOffsetOnAxis
from concourse.bass_utils import run_bass_kernel_spmd

F, VOCAB, D = 23, 100000, 64
B_FULL = 16384
NCORES = 8
BC = 64                        # samples per chunk
NB = BC * D // 128             # 32 row-blocks per chunk
NV = BC * D                    # 4096 columns per chunk
ETA = 2.0 ** -18
GATHER = "ind1"                # "ind1" | "quad"
f32 = mybir.dt.float32
i32 = mybir.dt.int32
i16 = mybir.dt.int16
u8 = mybir.dt.uint8
alu = mybir.AluOpType
AX = mybir.AxisListType
ACT = mybir.ActivationFunctionType

# (W, C, k, nrounds, kill_kept)
L1 = (28, 4, 20, 8, False)
L2 = (24, 4, 7, 7, True)
L3 = (9, 4, 3, 3, True)


def conv_matrix(w, W_in, kw, C_in, C_out, W_out):
    K = np.zeros((W_in * C_in, C_out * W_out), np.float32)
    for i in range(W_out):
        for j in range(kw):
            wsrc = i + j - (kw - 1)
            if 0 <= wsrc < W_in:
                for ci in range(C_in):
                    for co in range(C_out):
                        K[wsrc * C_in + ci, co * W_out + i] = w[j, ci, co]
    return K


def conv_matrix_cmajor_rows(w, W_in, kw, C_in, C_out, W_out):
    K = conv_matrix(w, W_in, kw, C_in, C_out, W_out)
    K = K.reshape(W_in, C_in, C_out * W_out).transpose(1, 0, 2)
    return np.ascontiguousarray(K.reshape(C_in * W_in, C_out * W_out))


def bcast_w(ap3, W):
    """append a stride-0 W axis to an AP."""
    return bass.AP(ap3.tensor, ap3.offset,
                   [list(p) for p in ap3.ap] + [[0, W]])


def bc_zb(zb, H, C, W):
    """zb [128,1] -> [128, H, C, W] stride-0 broadcast."""
    return bass.AP(zb.tensor, zb.offset,
                   [list(zb.ap[0]), [0, H], [0, C], [0, W]])


def build_program(b_core):
    nch = b_core // BC
    npair = nch // 2
    nc = bacc.Bacc(None, target_bir_lowering=False, debug=False)

    tab_d = nc.dram_tensor("tab", [F * VOCAB, D], f32, kind="ExternalInput")
    if GATHER == "ind1":
        gix_d = nc.dram_tensor("gix", [128, npair, F], i32,
                               kind="ExternalInput")
    else:
        gix_d = nc.dram_tensor("gix", [npair, 128, F, 4, 8], i16,
                               kind="ExternalInput")
        gmk_d = nc.dram_tensor("gmk", [npair, 128, F, 4], f32,
                               kind="ExternalInput")
    kc1_d = nc.dram_tensor("kc1", [23, 112], f32, kind="ExternalInput")
    kc2_d = nc.dram_tensor("kc2", [80, 96], f32, kind="ExternalInput")
    kc3_d = nc.dram_tensor("kc3", [28, 36], f32, kind="ExternalInput")
    be1_d = nc.dram_tensor("be1", [112, 1], f32, kind="ExternalInput")
    be2_d = nc.dram_tensor("be2", [96, 1], f32, kind="ExternalInput")
    be3_d = nc.dram_tensor("be3", [36, 1], f32, kind="ExternalInput")
    wdt_d = nc.dram_tensor("wdt", [128, 12], f32, kind="ExternalInput")
    bde_d = nc.dram_tensor("bde", [2, 1], f32, kind="ExternalInput")
    id_d = nc.dram_tensor("ident", [128, 128], f32, kind="ExternalInput")
    hm_d = nc.dram_tensor("hmask", [128, 2], f32, kind="ExternalInput")
    out_d = nc.dram_tensor("out", [b_core, 1], f32, kind="ExternalOutput")

    with tile.TileContext(nc) as tc:
        with tc.tile_pool(name="const", bufs=1) as cp:
            kc1 = cp.tile([23, 112], f32)
            nc.sync.dma_start(kc1[:], kc1_d[:])
            kc2 = cp.tile([80, 96], f32)
            nc.sync.dma_start(kc2[:], kc2_d[:])
            kc3 = cp.tile([28, 36], f32)
            nc.sync.dma_start(kc3[:], kc3_d[:])
            be1 = cp.tile([112, 1], f32)
            nc.sync.dma_start(be1[:], be1_d[:])
            be2 = cp.tile([96, 1], f32)
            nc.sync.dma_start(be2[:], be2_d[:])
            be3 = cp.tile([36, 1], f32)
            nc.sync.dma_start(be3[:], be3_d[:])
            wdt = cp.tile([128, 3, 4], f32)
            nc.sync.dma_start(wdt[:],
                              wdt_d[:].rearrange("p (t c) -> p t c", t=3))
            bde = cp.tile([2, 1], f32)
            nc.sync.dma_start(bde[:], bde_d[:])
            ident = cp.tile([128, 128], f32)
            nc.sync.dma_start(ident[:], id_d[:])
            hmask = cp.tile([128, 2], f32)
            nc.sync.dma_start(hmask[:], hm_d[:])
            zb = cp.tile([128, 1], f32)
            nc.vector.memset(zb[:], 0.0)
            nb1 = cp.tile([128, 1], f32)
            nc.vector.memset(nb1[:], -1.0)
            if GATHER == "ind1":
                gix = cp.tile([128, npair, F], i32)
                nc.sync.dma_start(gix[:], gix_d[:])
                gctx = (gix,)
            else:
                gctx = (gix_d, gmk_d)

            with tc.tile_pool(name="work", bufs=1) as wp, \
                 tc.tile_pool(name="gat", bufs=1) as gpool, \
                 tc.tile_pool(name="stg", bufs=2) as sp:
                ectx = {}
                if GATHER == "quad":
                    with nc.gpsimd.register("gnreg") as gnreg:
                        nc.gpsimd.reg_mov(gnreg, 128)
                        run_chunks(nc, tc, nch, wp, gpool, sp, tab_d, gctx,
                                   kc1, kc2, kc3, be1, be2, be3, wdt, bde,
                                   ident, hmask, zb, nb1, out_d, gnreg)
                else:
                    run_chunks(nc, tc, nch, wp, gpool, sp, tab_d, gctx,
                               kc1, kc2, kc3, be1, be2, be3, wdt, bde,
                               ident, hmask, zb, nb1, out_d, None)
    nc.compile()
    return nc


def run_chunks(nc, tc, nch, wp, gpool, sp, tab_d, gctx, kc1, kc2, kc3,
               be1, be2, be3, wdt, bde, ident, hmask, zb, nb1, out_d, gnreg):
    ecolP = gpool.tile([F, 128, D], f32, name="ecolP", tag="ecolP")
    erow = gpool.tile([128, F, D], f32, name="erow", tag="erow")
    if GATHER == "quad":
        e4 = gpool.tile([128, 4, F, D], f32, name="e4", tag="e4")
        gmsk = gpool.tile([128, F, 4], f32, name="gmsk", tag="gmsk")
        gidxp = gpool.tile([128, F, 4, 8], i16, name="gidxp", tag="gidxp")
    for ch in range(nch):
        pr = ch // 2
        if ch % 2 == 0:
            # ---- gather a 128-sample pair into row layout ----
            if GATHER == "ind1":
                (gix,) = gctx
                for f in range(F):
                    nc.gpsimd.indirect_dma_start(
                        out=erow[:, f, :], out_offset=None, in_=tab_d[:],
                        in_offset=IndirectOffsetOnAxis(
                            ap=gix[:, pr, f:f + 1], axis=0))
            else:
                gix_d, gmk_d = gctx
                nc.sync.dma_start(gidxp[:], gix_d[pr])
                nc.sync.dma_start(gmsk[:], gmk_d[pr])
                for q in range(4):
                    for f in range(F):
                        base = f * VOCAB + q * 25000
                        nc.gpsimd.dma_gather(
                            e4[:, q, f:f + 1, :],
                            tab_d[base:base + 25000, :],
                            gidxp[:, f, q, :], num_idxs=128,
                            num_idxs_reg=gnreg, elem_size=D)
                gm = gmsk[:]

                def mb(q):
                    return bass.AP(gm.tensor, gm.offset + q * gm.ap[2][0],
                                   [list(gm.ap[0]), list(gm.ap[1]), [0, D]])
                nc.vector.tensor_tensor(erow[:], e4[:, 0], mb(0),
                                        op=alu.mult)
                tmp = wp.tile([128, F, D], f32, name="emt", tag="emt")
                for q in range(1, 4):
                    nc.vector.tensor_tensor(tmp[:], e4[:, q], mb(q),
                                            op=alu.mult)
                    nc.vector.tensor_tensor(erow[:], erow[:], tmp[:],
                                            op=alu.add)
            # ---- transpose to column layout [F, b, h] ----
            with tc.tile_pool(name=f"pse_{pr & 1}", bufs=2,
                              space=bass.MemorySpace.PSUM) as ps:
                for h in range(D):
                    pte = ps.tile([F, 128], f32, name="pte", tag="pte")
                    nc.tensor.transpose(pte[:], erow[:, :, h], ident[:])
                    nc.scalar.copy(ecolP[:, :, h], pte[:])
        chunk(nc, tc, ch, wp, sp, ecolP, kc1, kc2, kc3, be1, be2, be3,
              wdt, bde, ident, hmask, zb, nb1, out_d)


def topk(nc, tc, wp, work, lay, zb, tag):
    """Iterated extraction: kill the segment max (set to exactly 0),
    `nrounds` times, in place on `work` [128, NB, C, W]."""
    W, C, k, nrounds, kill_kept = lay
    H = NB // 2
    m = wp.tile([128, NB, C], f32, name=f"m{tag}", tag=f"m{tag}")
    eqk = wp.tile([128, NB, C, W], u8, name=f"eqk{tag}", tag=f"eqk{tag}")
    halves = [(slice(0, H), nc.vector), (slice(H, NB), nc.vector)]
    for t in range(nrounds):
        for hs, eng in halves:
            nc.vector.tensor_reduce(m[:, hs], work[:, hs], axis=AX.X,
                                    op=alu.max)
            eng.tensor_tensor(eqk[:, hs], work[:, hs],
                              bcast_w(m[:, hs], W), op=alu.is_equal)
            nc.vector.copy_predicated(work[:, hs], eqk[:, hs],
                                      bc_zb(zb[:], H, C, W))
    # drop mask + unsegmented scan + segment bases, per half
    d = wp.tile([128, NB, C, W], u8, name=f"d{tag}", tag=f"d{tag}")
    incl = wp.tile([128, NB, C, W], f32, name=f"incl{tag}", tag=f"incl{tag}")
    spt = wp.tile([128, NB * C], f32, name=f"sp{tag}", tag=f"sp{tag}")
    dop = alu.is_gt if kill_kept else alu.is_equal
    nseg = H * C
    for hs, eng in halves:
        lo = 0 if hs.start == 0 else nseg
        sph = spt[:, lo:lo + nseg]
        nc.vector.tensor_scalar(d[:, hs], work[:, hs], 0.0, None, op0=dop)
        nc.vector.tensor_tensor_scan(
            incl[:, hs].rearrange("p a c w -> p (a c w)"),
            d[:, hs].rearrange("p a c w -> p (a c w)"),
            d[:, hs].rearrange("p a c w -> p (a c w)"),
            0.0, op0=alu.add, op1=alu.bypass)
        nc.vector.memset(sph[:, 0:1], 0.0)
        nc.vector.tensor_copy(
            sph[:, 1:nseg],
            incl[:, hs].rearrange("p a c w -> p (a c) w")[:, 0:nseg - 1,
                                                          W - 1])
    return incl, spt


def compact(nc, tc, wp, src, incl, spt, z, lay, tag):
    """z[t] = src[t+s] at positions where incl-in-segment == s, descending s
    (imposters overwritten by the true source)."""
    W, C, k, _, _ = lay
    H = NB // 2
    spc = [wp.tile([128, NB * C], f32, name=f"spc{tag}{i}", tag=f"spc{tag}{i}")
           for i in range(2)]
    eqm = [wp.tile([128, NB, C, k], u8, name=f"eqm{tag}{i}", tag=f"eqm{tag}{i}")
           for i in range(2)]
    halves = [(slice(0, H), nc.vector), (slice(H, NB), nc.vector)]
    nseg = H * C
    for s in range(W - k, -1, -1):
        pb = s & 1
        for hs, eng in halves:
            lo = 0 if hs.start == 0 else nseg
            nc.vector.tensor_scalar(spc[pb][:, lo:lo + nseg],
                                    spt[:, lo:lo + nseg], float(s), None,
                                    op0=alu.add)
            eng.tensor_tensor(
                eqm[pb][:, hs], incl[:, hs, :, s:s + k],
                bcast_w(spc[pb][:, lo:lo + nseg].rearrange(
                    "p (a c) -> p a c", a=H), k),
                op=alu.is_equal)
            nc.vector.copy_predicated(z[:, hs], eqm[pb][:, hs],
                                      src[:, hs, :, s:s + k])


def conv_block(nc, ps, sp, kc, bias, scale, zsrc, zbias, zscale, rn_out,
               ident, Min, Mout, tag):
    """One 512-column block chain: [flip-in from zsrc if given] -> matmul ->
    biased activation -> flip-out into rn_out row blocks."""
    pass


def chunk(nc, tc, ch, wp, sp, ecolP, kc1, kc2, kc3, be1, be2, be3,
          wdt, bde, ident, hmask, zb, nb1, out_d):
    half = (ch % 2) * 64
    rn1 = wp.tile([128, NB, 4, 28], f32, name="rn1", tag="rn1")
    ecv = ecolP[:, half:half + 64, :].rearrange("f b h -> f (b h)")
    with tc.tile_pool(name=f"ps1_{ch & 1}", bufs=2,
                      space=bass.MemorySpace.PSUM) as ps:
        for b in range(NV // 512):
            pm = ps.tile([112, 512], f32, name="pm1", tag="pm1")
            nc.tensor.matmul(pm[:], kc1[:], ecv[:, b * 512:(b + 1) * 512],
                             start=True, stop=True)
            rs = sp.tile([112, 512], f32, name="rc1s", tag="rc1s")
            nc.scalar.activation(rs[:], pm[:], ACT.Relu, bias=be1[:],
                                 scale=-1.0)
            for q in range(4):
                pt = ps.tile([128, 112], f32, name="pt1", tag="pt1")
                nc.tensor.transpose(pt[:], rs[:, q * 128:(q + 1) * 128],
                                    ident[:112, :112])
                nc.scalar.copy(rn1[:, b * 4 + q, :, :].rearrange(
                    "p c w -> p (c w)"), pt[:])

    # ---- L1 topk: kill the 8 dropped in place; zeros mark drops ----
    z1 = wp.tile([128, NB, 4, 20], f32, name="z1", tag="z1")
    nc.vector.memset(z1[:], 0.0)
    incl1, sp1 = topk(nc, tc, wp, rn1, L1, zb, "1")
    compact(nc, tc, wp, rn1, incl1, sp1, z1, L1, "1")

    # ---- flip z1 -> col, conv2, flip back ----
    rn2 = wp.tile([128, NB, 4, 24], f32, name="rn2", tag="rn2")
    z1v = z1[:].rearrange("p a c w -> p a (c w)")
    with tc.tile_pool(name=f"ps2_{ch & 1}", bufs=2,
                      space=bass.MemorySpace.PSUM) as ps:
        for b in range(NV // 512):
            zc = sp.tile([80, 512], f32, name="zc1", tag="zc1")
            for q in range(4):
                pz = ps.tile([80, 128], f32, name="pz1", tag="pz1")
                nc.tensor.transpose(pz[:], z1v[:, b * 4 + q, :], ident[:])
                nc.scalar.activation(zc[:, q * 128:(q + 1) * 128], pz[:],
                                     ACT.Relu, bias=1.0, scale=-1.0)
            pm = ps.tile([96, 512], f32, name="pm2", tag="pm2")
            nc.tensor.matmul(pm[:], kc2[:], zc[:], start=True, stop=True)
            rs = sp.tile([96, 512], f32, name="rc2s", tag="rc2s")
            nc.scalar.activation(rs[:], pm[:], ACT.Relu, bias=be2[:],
                                 scale=1.0)
            for q in range(4):
                pt = ps.tile([128, 96], f32, name="pt2", tag="pt2")
                nc.tensor.transpose(pt[:], rs[:, q * 128:(q + 1) * 128],
                                    ident[:96, :96])
                nc.scalar.copy(rn2[:, b * 4 + q, :, :].rearrange(
                    "p c w -> p (c w)"), pt[:])

    # ---- L2 topk on scratch copy (zeros mark the kept) ----
    wk2 = wp.tile([128, NB, 4, 24], f32, name="wk2", tag="wk2")
    nc.scalar.copy(wk2[:], rn2[:])
    z2 = wp.tile([128, NB, 4, 7], f32, name="z2", tag="z2")
    nc.vector.memset(z2[:], 0.0)
    incl2, sp2 = topk(nc, tc, wp, wk2, L2, zb, "2")
    compact(nc, tc, wp, rn2, incl2, sp2, z2, L2, "2")

    # ---- flip z2 -> col, conv3, flip back ----
    rn3 = wp.tile([128, NB, 4, 9], f32, name="rn3", tag="rn3")
    z2v = z2[:].rearrange("p a c w -> p a (c w)")
    with tc.tile_pool(name=f"ps3_{ch & 1}", bufs=2,
                      space=bass.MemorySpace.PSUM) as ps:
        for b in range(NV // 512):
            zc = sp.tile([28, 512], f32, name="zc2", tag="zc2")
            for q in range(4):
                pz = ps.tile([28, 128], f32, name="pz2", tag="pz2")
                nc.tensor.transpose(pz[:], z2v[:, b * 4 + q, :], ident[:])
                nc.scalar.activation(zc[:, q * 128:(q + 1) * 128], pz[:],
                                     ACT.Relu, bias=nb1[0:28, :], scale=1.0)
            pm = ps.tile([36, 512], f32, name="pm3", tag="pm3")
            nc.tensor.matmul(pm[:], kc3[:], zc[:], start=True, stop=True)
            rs = sp.tile([36, 512], f32, name="rc3s", tag="rc3s")
            nc.scalar.activation(rs[:], pm[:], ACT.Relu, bias=be3[:],
                                 scale=1.0)
            for q in range(4):
                pt = ps.tile([128, 36], f32, name="pt3", tag="pt3")
                nc.tensor.transpose(pt[:], rs[:, q * 128:(q + 1) * 128],
                                    ident[:36, :36])
                nc.scalar.copy(rn3[:, b * 4 + q, :, :].rearrange(
                    "p c w -> p (c w)"), pt[:])

    # ---- L3 top-3 absorbed into dense ----
    wk3 = wp.tile([128, NB, 4, 9], f32, name="wk3", tag="wk3")
    nc.scalar.copy(wk3[:], rn3[:])
    topk(nc, tc, wp, wk3, L3, zb, "3")
    k3f = wp.tile([128, NB, 4, 9], f32, name="k3f", tag="k3f")
    nc.vector.tensor_scalar(k3f[:], wk3[:], 0.0, None, op0=alu.is_equal)
    inclk = wp.tile([128, NB, 4, 9], f32, name="inclk", tag="inclk")
    nc.vector.tensor_tensor_scan(
        inclk[:].rearrange("p a c w -> p (a c w)"),
        k3f[:].rearrange("p a c w -> p (a c w)"),
        k3f[:].rearrange("p a c w -> p (a c w)"),
        0.0, op0=alu.add, op1=alu.bypass)
    spk = wp.tile([128, NB * 4], f32, name="spk", tag="spk")
    nc.vector.memset(spk[:, 0:1], 0.0)
    nc.vector.tensor_copy(
        spk[:, 1:NB * 4],
        inclk[:].rearrange("p a c w -> p (a c) w")[:, 0:NB * 4 - 1, 8])
    wsel = wp.tile([128, NB, 4, 9], f32, name="wsel", tag="wsel")
    nc.vector.memset(wsel[:], 0.0)
    spc3 = wp.tile([128, NB * 4], f32, name="spc3", tag="spc3")
    eq3 = wp.tile([128, NB, 4, 9], u8, name="eq3", tag="eq3")
    for t in range(3):
        nc.vector.tensor_scalar(spc3[:], spk[:], float(t + 1), None,
                                op0=alu.add)
        nc.vector.tensor_tensor(
            eq3[:], inclk[:],
            bcast_w(spc3[:].rearrange("p (a c) -> p a c", a=NB), 9),
            op=alu.is_equal)
        wt = wdt[:, t, :]  # [128, 4]
        wtb = bass.AP(wt.tensor, wt.offset,
                      [list(wt.ap[0]), [0, NB], list(wt.ap[1]), [0, 9]])
        nc.vector.copy_predicated(wsel[:], eq3[:], wtb)
    q3 = wp.tile([128, NB, 4, 9], f32, name="q3", tag="q3")
    nc.scalar.activation(q3[:], rn3[:], ACT.Relu, bias=nb1[:], scale=1.0)
    nc.vector.tensor_tensor(q3[:], q3[:], wsel[:], op=alu.mult)
    nc.vector.tensor_tensor(q3[:], q3[:], k3f[:], op=alu.mult)
    dots = wp.tile([128, NB], f32, name="dots", tag="dots")
    nc.vector.tensor_reduce(dots[:], q3[:], axis=AX.XY, op=alu.add)
    osb = wp.tile([2, NB], f32, name="osb", tag="osb")
    with tc.tile_pool(name=f"pso_{ch & 1}", bufs=2,
                      space=bass.MemorySpace.PSUM) as ps:
        po = ps.tile([2, NB], f32, name="po", tag="po")
        nc.tensor.matmul(po[:], hmask[:], dots[:], start=True, stop=True)
        nc.scalar.activation(osb[:], po[:], ACT.Sigmoid, bias=bde[:],
                             scale=1.0)
    nc.sync.dma_start(
        out_d[ch * BC:(ch + 1) * BC, :].rearrange("(j m) o -> m (j o)", m=2),
        osb[:])


def host_prep(inputs, b_core):
    ids = np.asarray(inputs["ids"])
    tab = np.ascontiguousarray(
        np.asarray(inputs["emb_table"], dtype=np.float32).reshape(F * VOCAB, D))
    w1 = np.asarray(inputs["w1"], np.float32)[0]
    w2 = np.asarray(inputs["w2"], np.float32)[0]
    w3 = np.asarray(inputs["w3"], np.float32)[0]
    b1 = np.asarray(inputs["b1"], np.float32)
    b2 = np.asarray(inputs["b2"], np.float32)
    b3 = np.asarray(inputs["b3"], np.float32)
    kc1 = np.ascontiguousarray(-conv_matrix(w1, 23, 6, 1, 4, 28))
    kc2 = conv_matrix_cmajor_rows(w2, 20, 5, 4, 4, 24)
    kc3 = conv_matrix_cmajor_rows(w3, 7, 3, 4, 4, 9)
    i28 = np.tile(np.arange(28), 4)
    i24 = np.tile(np.arange(24), 4)
    i9 = np.tile(np.arange(9), 4)
    be1 = (1.0 - np.repeat(b1, 28) + (i28 + 1) * ETA).astype(np.float32)[:, None]
    be2 = (1.0 + np.repeat(b2, 24) + (24 - i24) * ETA).astype(np.float32)[:, None]
    be3 = (1.0 + np.repeat(b3, 9) + (9 - i9) * ETA).astype(np.float32)[:, None]
    wd = np.asarray(inputs["wd"], np.float32).reshape(D, 12)
    wdt = wd[np.arange(128) % D].copy()
    bde = np.full((2, 1), np.asarray(inputs["bd"], np.float32).ravel()[0],
                  np.float32)
    ident = np.eye(128, dtype=np.float32)
    hmask = np.zeros((128, 2), np.float32)
    hmask[np.arange(128), np.arange(128) // 64] = 1.0

    npair = b_core // 128
    in_maps = []
    for c in range(NCORES):
        idsc = ids[c * b_core:(c + 1) * b_core].astype(np.int64)  # [b, F]
        im = {"tab": tab, "kc1": kc1, "kc2": kc2, "kc3": kc3, "be1": be1,
              "be2": be2, "be3": be3, "wdt": wdt, "bde": bde, "ident": ident,
              "hmask": hmask}
        if GATHER == "ind1":
            gix = (np.arange(F)[None, None, :] * VOCAB +
                   idsc.reshape(npair, 128, F).transpose(1, 0, 2)
                   ).astype(np.int32)
            im["gix"] = np.ascontiguousarray(gix)      # [128, npair, F]
        else:
            idc = idsc.reshape(npair, 128, F)
            gidx = np.zeros((npair, 128, F, 4, 8), np.int16)
            gmsk = np.zeros((npair, 128, F, 4), np.float32)
            for q in range(4):
                lo = q * 25000
                sub = np.clip(idc - lo, 0, 24999).astype(np.int16)
                wr = np.zeros((npair, 128, F, 8), np.int16)
                for j in range(128):
                    wr[:, j % 16, :, j // 16] = sub[:, j, :]
                gidx[:, :, :, q, :] = np.tile(wr[:, :16], (1, 8, 1, 1))
                gmsk[:, :, :, q] = (idc >= lo) & (idc < lo + 25000)
            im["gix"] = gidx
            im["gmk"] = gmsk
        in_maps.append(im)
    return in_maps


def kernel(**inputs):
    b_core = np.asarray(inputs["ids"]).shape[0] // NCORES
    nc = build_program(b_core)
    in_maps = host_prep(inputs, b_core)
    res = run_bass_kernel_spmd(nc, in_maps, list(range(NCORES)))
    outs = [np.asarray(r["out"]).reshape(b_core, 1) for r in res.results]
    return np.concatenate(outs, axis=0).astype(np.float32)
